# revision 26
# baseline (speedup 1.0000x reference)
"""Trainium2 Bass kernel for causal bilinear self-attention (diagonal variant).

Computes, per (b, head):
    scores[t, s] = h[b, t] @ A[head] @ h[b, s]        (causal: s <= t)
    attn = softmax(scores, axis=-1)
    out[b, head, t, :] = attn[t, t] * h[b, t, :]
returned reshaped row-major to (B, T, H*d)  (faithful torch .view semantics).

Only the diagonal of the attention matrix is needed:
    attn[t, t] = exp(scores[t,t] - m) / sum_{s<=t} exp(scores[t,s] - m)
               = 1 / sum_{s<=t} exp(scores[t,s] - scores[t,t])
Using bias = -scores[t,t] inside the exp (instead of the row max) keeps the
denominator in [1, inf) -- the s==t term is exactly exp(0) -- so NO row-max
pass is needed: overflow to inf gives reciprocal 0, matching the true
underflowed attention weight.  This removes the reduce_max over the whole
causal triangle and the exp-diagonal extraction, which made the vector
engine the bottleneck (86% busy vs PE 64%) in the max-subtracting version.

Engine budget per core (r1, cost model): PE ~92 us (bound), DVE ~80 us,
ACT ~59 us.  Layout/scheduling choices:
  - h[b] stays resident in SBUF (4 MB): loaded once, reused for transposes
    and the final attn*h scale (no reload).
  - A DMA'd per head so stage 1 of head 0 isn't gated on both heads' A.
  - hT via PE transpose with f32r-tagged input (1.5 cyc/row vs 2.0 for f32;
    the f32r mantissa rounding is applied by the PE at matmul read anyway).
  - head 0 walks row tiles ascending, head 1 descending, with stage-1
    t-slices ordered to match, so the final row tile is the 1-chunk tile 0
    (short epilogue tail instead of the 4-chunk tile 15).
  - per row tile, the diagonal chunk's matmuls are emitted FIRST so the
    bias (-scores[t,t]) is ready before the other chunks' exps; the
    lsum/reciprocal/scale epilogue of tile k is emitted after tile k+1's
    exps so the in-order DVE never stalls waiting on ACT.

Precision: TensorE fp32 costs 4 cyc/row; float32r (TF32-like, ~11-bit
mantissa) costs 1 cyc/row at moving dim >= 256.  "r1" = single f32r pass
per stage (measured 3.4e-3 rel err on HW vs the 2e-2 gate).  "r2" adds a
residual pass for A (stage 1) / g (stage 2); "f32" is the exact path.

Hardware notes (found empirically on this axon/neuronxcc toolchain):
  - tensor_tensor_reduce with a PSUM input crashes the device; so does an
    ACT read of a PSUM region modified in place by the DVE.  PSUM is
    written only by the PE; DVE copy/reduce-class ops and ACT activations
    may read it; two-operand DVE ops only run on SBUF.
  - mask constants are DMA'd from host inputs (no gpsimd affine_select).

Sharding: 16 (b, head) pairs across 8 cores -> core c handles b = c // 4,
heads 2*(c%4) and 2*(c%4)+1.
"""

import os
import sys

try:
    import concourse.bass  # noqa: F401
except ImportError:  # pragma: no cover
    sys.path.insert(0, "/opt/trn_rl_repo")

import numpy as np

import concourse.bass as bass  # noqa: F401
import concourse.tile as tile
from concourse import bacc, bass_utils, mybir

B, T, D, H = 2, 2048, 512, 8
NCORES = 8
P = 128
NT = T // P      # 16 row tiles
ND = D // P      # 4 contraction chunks
SCH = 512        # s-chunk width (one PSUM bank of fp32)
NS = T // SCH    # 4 column slices of hT
NEG = -1.0e30

f32 = mybir.dt.float32
f32r = mybir.dt.float32r

STAGE1 = os.environ.get("BK_STAGE1", "r1")
STAGE2 = os.environ.get("BK_STAGE2", "r1")


def build_nc(stage1=None, stage2=None):
    stage1 = stage1 or STAGE1
    stage2 = stage2 or STAGE2
    assert stage1 in ("f32", "r1", "r2") and stage2 in ("f32", "r1", "r2")
    s1_r = stage1 != "f32"
    s2_r = stage2 != "f32"
    need_hT32 = stage1 == "f32" or stage2 == "f32"
    need_hTr = s1_r or s2_r

    nc = bacc.Bacc("TRN2", target_bir_lowering=False, debug=False)
    hb = nc.dram_tensor("hb", [T, D], f32, kind="ExternalInput")
    A2 = nc.dram_tensor("A2", [2, D, D], f32, kind="ExternalInput")
    cmaskd = nc.dram_tensor("cmaskd", [P, P], f32, kind="ExternalInput")
    identd = nc.dram_tensor("identd", [P, P], f32, kind="ExternalInput")
    out2 = nc.dram_tensor("out2", [2, T, D], f32, kind="ExternalOutput")
    hb_t = hb[:].rearrange("(n p) d -> p n d", p=P)  # [128, 16, 512] view

    AX = mybir.AxisListType.X
    EXP = mybir.ActivationFunctionType.Exp

    with tile.TileContext(nc) as tc:
        with (
            tc.tile_pool(name="const", bufs=1) as constp,
            tc.tile_pool(name="big", bufs=1) as big,
            tc.tile_pool(name="gpool", bufs=1) as gpool,
            tc.tile_pool(name="psum", bufs=8, space="PSUM") as psum,
            tc.tile_pool(name="scs", bufs=3) as scs,
            tc.tile_pool(name="escr", bufs=2) as escr,
            tc.tile_pool(name="stats", bufs=10) as stats,
            tc.tile_pool(name="outp", bufs=3) as outp,
        ):
            ident = constp.tile([P, P], f32)
            nc.gpsimd.dma_start(out=ident, in_=identd[:])
            cmask = constp.tile([P, P], f32)
            nc.gpsimd.dma_start(out=cmask, in_=cmaskd[:])

            # h resident in SBUF: hres[p, i, d] = h[i*128 + p, d].  Tile 0 is
            # split per 128-col chunk so the first transpose starts sooner,
            # and tiles round-robin over three DGE queues so arrival is not
            # paced by one queue's dispatch rate.  A's per-head DMAs slot
            # into the gpsimd queue: A0 early enough to be rounded before
            # stage 1 needs it (~10us), A1 at the back (needed ~45us).
            A_sb = big.tile([P, 2, ND, D], f32)
            hres = big.tile([P, NT, D], f32)
            for c in range(ND):
                nc.sync.dma_start(
                    out=hres[:, 0, c * P : (c + 1) * P],
                    in_=hb_t[:, 0, c * P : (c + 1) * P],
                )
            dmaq = [nc.sync, nc.scalar, nc.gpsimd]
            for i in range(1, NT):
                dmaq[i % 3].dma_start(out=hres[:, i, :], in_=hb_t[:, i, :])
                if i == 5:  # after gpsimd has queued h2, h5
                    nc.gpsimd.dma_start(
                        out=A_sb[:, 0],
                        in_=A2[0].rearrange("(c p) e -> p c e", p=P),
                    )
            nc.gpsimd.dma_start(
                out=A_sb[:, 1], in_=A2[1].rearrange("(c p) e -> p c e", p=P)
            )
            if s1_r:
                # BIR verifier requires f32r matmul inputs to come from an
                # op that rounds to f32r, so an explicit rounded copy (a
                # bitcast view of the DMA'd f32 tile is rejected).  The
                # copies run on the otherwise-idle Pool engine so the DVE
                # hT drains aren't delayed behind them.
                A_r = big.tile([P, 2, ND, D], f32r)
                for hd_ in range(2):
                    for dc in range(ND):
                        nc.gpsimd.tensor_copy(
                            A_r[:, hd_, dc], A_sb[:, hd_, dc]
                        )

            def a_hi(hd, dc, ecs):
                if s1_r:
                    return A_r[:, hd, dc, ecs]
                return A_sb[:, hd, dc, ecs]

            # h^T: hT*[p, c, t] = h[t, c*128 + p], via PE transpose
            def _mk(name, dt_):
                return [[big.tile([P, SCH], dt_, name=f"{name}_{c}_{s}")
                         for s in range(NS)] for c in range(ND)]
            hT32 = _mk("hT32", f32) if need_hT32 else None
            hTr = _mk("hTr", f32r) if need_hTr else None

            def _hT(tens, c, lo, width):
                s, off = lo // SCH, lo % SCH
                return tens[c][s][:, off : off + width]

            need_g32 = stage2 == "f32"

            if stage1 == "f32":
                s1_rhs = [hT32]
            else:
                s1_rhs = [hTr] if stage1 == "r1" else [hTr, hTr]
            n1 = (2 if stage1 == "r2" else 1) * ND

            def emit_stage1_tsl(hd, tsl, g32, gh, gl):
                ts_ = slice(tsl * SCH, (tsl + 1) * SCH)
                for ec in range(ND):
                    ecs = slice(ec * P, (ec + 1) * P)
                    pg = psum.tile([P, SCH], f32, tag="ps")
                    k = 0
                    for ip in range(2 if stage1 == "r2" else 1):
                        for dc in range(ND):
                            lhs = (A_l[:, hd, dc, ecs] if (stage1 == "r2" and ip == 1)
                                   else a_hi(hd, dc, ecs))
                            nc.tensor.matmul(
                                pg, lhs, s1_rhs[ip][dc][tsl],
                                start=(k == 0), stop=(k == n1 - 1),
                            )
                            k += 1
                    if g32 is not None:
                        nc.vector.tensor_copy(g32[:, ec, ts_], pg)
                    if gh is not None:
                        nc.vector.tensor_copy(gh[:, ec, ts_], pg)
                    if gl is not None:
                        nc.vector.tensor_sub(
                            gl[:, ec, ts_], pg, gh[:, ec, ts_].bitcast(f32)
                        )

            def alloc_g():
                g32 = gpool.tile([P, ND, T], f32, tag="g32", name="gT32") if need_g32 else None
                gh = gpool.tile([P, ND, T], f32r, tag="gh", name="gTh") if s2_r else None
                gl = gpool.tile([P, ND, T], f32r, tag="gl", name="gTl") if stage2 == "r2" else None
                return g32, gh, gl

            for i in range(NT):
                for c in range(ND):
                    src = hres[:, i, c * P : (c + 1) * P]
                    pt = psum.tile([P, P], f32, tag="ps")
                    nc.tensor.transpose(pt, src, ident)
                    if need_hT32:
                        nc.vector.tensor_copy(_hT(hT32, c, i * P, P), pt)
                    if need_hTr:
                        nc.vector.tensor_copy(_hT(hTr, c, i * P, P), pt)
            if stage1 == "r2":
                A_l = big.tile([P, 2, ND, D], f32r)
                nc.vector.tensor_sub(A_l, A_sb, A_r.bitcast(f32))

            g_head0 = alloc_g()
            for tsl in range(NS):
                emit_stage1_tsl(0, tsl, *g_head0)

            for hd in range(2):
                # ascending everywhere: the big late tiles keep the PE busy
                # while earlier tiles' epilogue chains drain.  Head 1 ends on
                # small tile 3 so only a short chain trails the last matmul.
                if hd == 0:
                    gT32, gTh, gTl = g_head0
                    tile_order = list(range(NT))
                else:
                    gT32, gTh, gTl = alloc_g()
                    for tsl in range(NS):
                        emit_stage1_tsl(1, tsl, gT32, gTh, gTl)
                    tile_order = list(range(NT))

                if stage2 == "f32":
                    s2_passes = [(gT32, hT32)]
                elif stage2 == "r1":
                    s2_passes = [(gTh, hTr)]
                else:
                    s2_passes = [(gTh, hTr), (gTl, hTr)]
                n2 = len(s2_passes) * ND

                # ---- stage 2 + diag-bias softmax, per row tile ----
                pend = None  # deferred epilogue: (lp, nch, i)

                def flush(pend):
                    lp, nch, i = pend
                    its = slice(i * P, (i + 1) * P)
                    lsum = stats.tile([P, 1], f32, tag="ls")
                    nc.vector.reduce_sum(out=lsum, in_=lp[:, :nch], axis=AX)
                    rl = stats.tile([P, 1], f32, tag="rl")
                    nc.vector.reciprocal(rl, lsum)
                    ot = outp.tile([P, D], f32, tag="ot")
                    nc.vector.tensor_scalar_mul(ot, hres[:, i, :], rl)
                    nc.sync.dma_start(out=out2[hd, its, :], in_=ot)

                for i in tile_order:
                    nch = i // 4 + 1
                    its = slice(i * P, (i + 1) * P)
                    dcol = (i % 4) * P       # diag block start within last chunk
                    wlast = dcol + P         # causal width of last chunk
                    # f32r matmuls need moving dim >= 256 for full rate; widen
                    # (extra cols never read out of PSUM)
                    w_mm = max(wlast, 2 * P) if s2_r else wlast
                    jlast = nch - 1

                    # diag chunk first: its matmuls feed the bias every other
                    # chunk's exp needs
                    psD = psum.tile([P, SCH], f32, tag="ps")
                    k = 0
                    for lhs_src, rhs_src in s2_passes:
                        for ec in range(ND):
                            nc.tensor.matmul(
                                psD[:, :w_mm],
                                lhs_src[:, ec, its],
                                rhs_src[ec][jlast][:, :w_mm],
                                start=(k == 0), stop=(k == n2 - 1),
                            )
                            k += 1
                    sc = scs.tile([P, SCH], f32, tag="sc")
                    nc.vector.tensor_copy(sc[:, :wlast], psD[:, :wlast])
                    # diag extraction: mul by identity + negated row-sum
                    # (tensor_tensor_reduce crashes the device on this
                    # toolchain even with SBUF-only operands).  The mul and
                    # the causal-mask add run on the otherwise-idle Pool
                    # (gpsimd) engine; the X-axis reduce must stay on DVE.
                    dscr = stats.tile([P, P], f32, tag="dscr")
                    nc.vector.tensor_mul(dscr, sc[:, dcol : dcol + P], ident)
                    negdiag = stats.tile([P, 1], f32, tag="nd")
                    nc.vector.reduce_sum(
                        out=negdiag, in_=dscr, axis=AX, negate=True
                    )
                    nc.gpsimd.tensor_add(
                        sc[:, dcol : dcol + P], sc[:, dcol : dcol + P], cmask
                    )
                    lp = stats.tile([P, 4], f32, tag="lp")
                    for j in range(nch - 1):
                        ps = psum.tile([P, SCH], f32, tag="ps")
                        k = 0
                        for lhs_src, rhs_src in s2_passes:
                            for ec in range(ND):
                                nc.tensor.matmul(
                                    ps,
                                    lhs_src[:, ec, its],
                                    rhs_src[ec][j],
                                    start=(k == 0), stop=(k == n2 - 1),
                                )
                                k += 1
                        ex = escr.tile([P, SCH], f32, tag="ex")
                        nc.scalar.activation(
                            out=ex, in_=ps, func=EXP,
                            bias=negdiag, scale=1.0,
                            accum_out=lp[:, j : j + 1],
                        )
                    exd = escr.tile([P, SCH], f32, tag="ex")
                    nc.scalar.activation(
                        out=exd[:, :wlast], in_=sc[:, :wlast], func=EXP,
                        bias=negdiag, scale=1.0,
                        accum_out=lp[:, jlast : jlast + 1],
                    )

                    if pend is not None:
                        flush(pend)
                    pend = (lp, nch, i)
                flush(pend)

    nc.compile()
    return nc


_NC_CACHE = {}


def _get_nc(stage1=None, stage2=None):
    key = (stage1 or STAGE1, stage2 or STAGE2)
    if key not in _NC_CACHE:
        _NC_CACHE[key] = build_nc(*key)
    return _NC_CACHE[key]


def _consts():
    cmask = np.triu(np.full((P, P), NEG, np.float32), 1)
    ident = np.eye(P, dtype=np.float32)
    return cmask, ident


def make_in_maps(h, A):
    h = np.ascontiguousarray(h, dtype=np.float32)
    A = np.ascontiguousarray(A, dtype=np.float32)
    cmask, ident = _consts()
    in_maps = []
    for c in range(NCORES):
        b = c // 4
        h0 = 2 * (c % 4)
        in_maps.append({"hb": h[b], "A2": np.ascontiguousarray(A[h0 : h0 + 2]),
                        "cmaskd": cmask, "identd": ident})
    return in_maps


def assemble(results):
    full = np.empty((B, H, T, D), dtype=np.float32)
    for c in range(NCORES):
        b = c // 4
        h0 = 2 * (c % 4)
        o = results[c]["out2"]
        full[b, h0] = o[0]
        full[b, h0 + 1] = o[1]
    return full.reshape(B, T, H * D)


def kernel(h, A):
    nc = _get_nc()
    res = bass_utils.run_bass_kernel_spmd(
        nc, make_in_maps(h, A), core_ids=list(range(NCORES))
    )
    return assemble(res.results)


# revision 28
# speedup vs baseline: 1.0241x; 1.0241x over previous
"""Trainium2 Bass kernel for causal bilinear self-attention (diagonal variant).

Computes, per (b, head):
    scores[t, s] = h[b, t] @ A[head] @ h[b, s]        (causal: s <= t)
    attn = softmax(scores, axis=-1)
    out[b, head, t, :] = attn[t, t] * h[b, t, :]
returned reshaped row-major to (B, T, H*d)  (faithful torch .view semantics).

Only the diagonal of the attention matrix is needed:
    attn[t, t] = exp(scores[t,t] - m) / sum_{s<=t} exp(scores[t,s] - m)
               = 1 / sum_{s<=t} exp(scores[t,s] - scores[t,t])
Using bias = -scores[t,t] inside the exp (instead of the row max) keeps the
denominator in [1, inf) -- the s==t term is exactly exp(0) -- so NO row-max
pass is needed: overflow to inf gives reciprocal 0, matching the true
underflowed attention weight.  This removes the reduce_max over the whole
causal triangle and the exp-diagonal extraction, which made the vector
engine the bottleneck (86% busy vs PE 64%) in the max-subtracting version.

Engine budget per core (r1, cost model): PE ~92 us (bound), DVE ~80 us,
ACT ~59 us.  Layout/scheduling choices:
  - h[b] stays resident in SBUF (4 MB): loaded once, reused for transposes
    and the final attn*h scale (no reload).
  - A DMA'd per head so stage 1 of head 0 isn't gated on both heads' A.
  - hT via PE transpose with f32r-tagged input (1.5 cyc/row vs 2.0 for f32;
    the f32r mantissa rounding is applied by the PE at matmul read anyway).
  - head 0 walks row tiles ascending, head 1 descending, with stage-1
    t-slices ordered to match, so the final row tile is the 1-chunk tile 0
    (short epilogue tail instead of the 4-chunk tile 15).
  - per row tile, the diagonal chunk's matmuls are emitted FIRST so the
    bias (-scores[t,t]) is ready before the other chunks' exps; the
    lsum/reciprocal/scale epilogue of tile k is emitted after tile k+1's
    exps so the in-order DVE never stalls waiting on ACT.

Precision: TensorE fp32 costs 4 cyc/row; float32r (TF32-like, ~11-bit
mantissa) costs 1 cyc/row at moving dim >= 256.  "r1" = single f32r pass
per stage (measured 3.4e-3 rel err on HW vs the 2e-2 gate).  "r2" adds a
residual pass for A (stage 1) / g (stage 2); "f32" is the exact path.

Hardware notes (found empirically on this axon/neuronxcc toolchain):
  - tensor_tensor_reduce with a PSUM input crashes the device; so does an
    ACT read of a PSUM region modified in place by the DVE.  PSUM is
    written only by the PE; DVE copy/reduce-class ops and ACT activations
    may read it; two-operand DVE ops only run on SBUF.
  - mask constants are DMA'd from host inputs (no gpsimd affine_select).

Sharding: 16 (b, head) pairs across 8 cores -> core c handles b = c // 4,
heads 2*(c%4) and 2*(c%4)+1.
"""

import os
import sys

try:
    import concourse.bass  # noqa: F401
except ImportError:  # pragma: no cover
    sys.path.insert(0, "/opt/trn_rl_repo")

import numpy as np

import concourse.bass as bass  # noqa: F401
import concourse.tile as tile
from concourse import bacc, bass_utils, mybir

B, T, D, H = 2, 2048, 512, 8
NCORES = 8
P = 128
NT = T // P      # 16 row tiles
ND = D // P      # 4 contraction chunks
SCH = 512        # s-chunk width (one PSUM bank of fp32)
NS = T // SCH    # 4 column slices of hT
NEG = -1.0e30

f32 = mybir.dt.float32
f32r = mybir.dt.float32r

STAGE1 = os.environ.get("BK_STAGE1", "r1")
STAGE2 = os.environ.get("BK_STAGE2", "r1")


def build_nc(stage1=None, stage2=None):
    stage1 = stage1 or STAGE1
    stage2 = stage2 or STAGE2
    assert stage1 in ("f32", "r1", "r2") and stage2 in ("f32", "r1", "r2")
    s1_r = stage1 != "f32"
    s2_r = stage2 != "f32"
    need_hT32 = stage1 == "f32" or stage2 == "f32"
    need_hTr = s1_r or s2_r

    nc = bacc.Bacc("TRN2", target_bir_lowering=False, debug=False)
    hb = nc.dram_tensor("hb", [T, D], f32, kind="ExternalInput")
    A2 = nc.dram_tensor("A2", [2, D, D], f32, kind="ExternalInput")
    cmaskd = nc.dram_tensor("cmaskd", [P, P], f32, kind="ExternalInput")
    identd = nc.dram_tensor("identd", [P, P], f32, kind="ExternalInput")
    out2 = nc.dram_tensor("out2", [2, T, D], f32, kind="ExternalOutput")
    hb_t = hb[:].rearrange("(n p) d -> p n d", p=P)  # [128, 16, 512] view

    AX = mybir.AxisListType.X
    EXP = mybir.ActivationFunctionType.Exp

    with tile.TileContext(nc) as tc:
        with (
            tc.tile_pool(name="const", bufs=1) as constp,
            tc.tile_pool(name="big", bufs=1) as big,
            tc.tile_pool(name="gpool", bufs=1) as gpool,
            tc.tile_pool(name="psum", bufs=8, space="PSUM") as psum,
            tc.tile_pool(name="scs", bufs=3) as scs,
            tc.tile_pool(name="escr", bufs=2) as escr,
            tc.tile_pool(name="stats", bufs=10) as stats,
            tc.tile_pool(name="outp", bufs=3) as outp,
        ):
            ident = constp.tile([P, P], f32)
            nc.gpsimd.dma_start(out=ident, in_=identd[:])
            cmask = constp.tile([P, P], f32)
            nc.gpsimd.dma_start(out=cmask, in_=cmaskd[:])

            # h resident in SBUF: hres[p, i, d] = h[i*128 + p, d].  Tile 0 is
            # split per 128-col chunk so the first transpose starts sooner,
            # and tiles round-robin over three DGE queues so arrival is not
            # paced by one queue's dispatch rate.  A's per-head DMAs slot
            # into the gpsimd queue: A0 early enough to be rounded before
            # stage 1 needs it (~10us), A1 at the back (needed ~45us).
            A_sb = big.tile([P, 2, ND, D], f32)
            hres = big.tile([P, NT, D], f32)
            for c in range(ND):
                nc.sync.dma_start(
                    out=hres[:, 0, c * P : (c + 1) * P],
                    in_=hb_t[:, 0, c * P : (c + 1) * P],
                )
            dmaq = [nc.sync, nc.scalar, nc.gpsimd]
            for i in range(1, NT):
                dmaq[i % 3].dma_start(out=hres[:, i, :], in_=hb_t[:, i, :])
            for hd in range(2):
                nc.gpsimd.dma_start(
                    out=A_sb[:, hd],
                    in_=A2[hd].rearrange("(c p) e -> p c e", p=P),
                )
            if s1_r:
                # BIR verifier requires f32r matmul inputs to come from an
                # op that rounds to f32r, so an explicit rounded copy (a
                # bitcast view of the DMA'd f32 tile is rejected).  The
                # copies run on the otherwise-idle Pool engine so the DVE
                # hT drains aren't delayed behind them.
                A_r = big.tile([P, 2, ND, D], f32r)
                for hd_ in range(2):
                    for dc in range(ND):
                        nc.gpsimd.tensor_copy(
                            A_r[:, hd_, dc], A_sb[:, hd_, dc]
                        )

            def a_hi(hd, dc, ecs):
                if s1_r:
                    return A_r[:, hd, dc, ecs]
                return A_sb[:, hd, dc, ecs]

            # h^T: hT*[p, c, t] = h[t, c*128 + p], via PE transpose
            def _mk(name, dt_):
                return [[big.tile([P, SCH], dt_, name=f"{name}_{c}_{s}")
                         for s in range(NS)] for c in range(ND)]
            hT32 = _mk("hT32", f32) if need_hT32 else None
            hTr = _mk("hTr", f32r) if need_hTr else None

            def _hT(tens, c, lo, width):
                s, off = lo // SCH, lo % SCH
                return tens[c][s][:, off : off + width]

            need_g32 = stage2 == "f32"

            if stage1 == "f32":
                s1_rhs = [hT32]
            else:
                s1_rhs = [hTr] if stage1 == "r1" else [hTr, hTr]
            n1 = (2 if stage1 == "r2" else 1) * ND

            def emit_stage1_tsl(hd, tsl, g32, gh, gl):
                ts_ = slice(tsl * SCH, (tsl + 1) * SCH)
                for ec in range(ND):
                    ecs = slice(ec * P, (ec + 1) * P)
                    pg = psum.tile([P, SCH], f32, tag="ps")
                    k = 0
                    for ip in range(2 if stage1 == "r2" else 1):
                        for dc in range(ND):
                            lhs = (A_l[:, hd, dc, ecs] if (stage1 == "r2" and ip == 1)
                                   else a_hi(hd, dc, ecs))
                            nc.tensor.matmul(
                                pg, lhs, s1_rhs[ip][dc][tsl],
                                start=(k == 0), stop=(k == n1 - 1),
                            )
                            k += 1
                    if g32 is not None:
                        nc.vector.tensor_copy(g32[:, ec, ts_], pg)
                    if gh is not None:
                        nc.vector.tensor_copy(gh[:, ec, ts_], pg)
                    if gl is not None:
                        nc.vector.tensor_sub(
                            gl[:, ec, ts_], pg, gh[:, ec, ts_].bitcast(f32)
                        )

            def alloc_g():
                g32 = gpool.tile([P, ND, T], f32, tag="g32", name="gT32") if need_g32 else None
                gh = gpool.tile([P, ND, T], f32r, tag="gh", name="gTh") if s2_r else None
                gl = gpool.tile([P, ND, T], f32r, tag="gl", name="gTl") if stage2 == "r2" else None
                return g32, gh, gl

            for i in range(NT):
                for c in range(ND):
                    src = hres[:, i, c * P : (c + 1) * P]
                    pt = psum.tile([P, P], f32, tag="ps")
                    nc.tensor.transpose(pt, src, ident)
                    if need_hT32:
                        nc.vector.tensor_copy(_hT(hT32, c, i * P, P), pt)
                    if need_hTr:
                        nc.vector.tensor_copy(_hT(hTr, c, i * P, P), pt)
            if stage1 == "r2":
                A_l = big.tile([P, 2, ND, D], f32r)
                nc.vector.tensor_sub(A_l, A_sb, A_r.bitcast(f32))

            g_head0 = alloc_g()
            for tsl in range(NS):
                emit_stage1_tsl(0, tsl, *g_head0)

            for hd in range(2):
                # ascending everywhere: the big late tiles keep the PE busy
                # while earlier tiles' epilogue chains drain.  Head 1 ends on
                # small tile 3 so only a short chain trails the last matmul.
                if hd == 0:
                    gT32, gTh, gTl = g_head0
                    tile_order = list(range(NT))
                else:
                    gT32, gTh, gTl = alloc_g()
                    for tsl in range(NS):
                        emit_stage1_tsl(1, tsl, gT32, gTh, gTl)
                    tile_order = [0, 1, 2] + list(range(4, NT)) + [3]

                if stage2 == "f32":
                    s2_passes = [(gT32, hT32)]
                elif stage2 == "r1":
                    s2_passes = [(gTh, hTr)]
                else:
                    s2_passes = [(gTh, hTr), (gTl, hTr)]
                n2 = len(s2_passes) * ND

                # ---- stage 2 + diag-bias softmax, per row tile ----
                pend = None  # deferred epilogue: (lp, nch, i)

                def flush(pend):
                    lp, nch, i = pend
                    its = slice(i * P, (i + 1) * P)
                    lsum = stats.tile([P, 1], f32, tag="ls")
                    nc.vector.reduce_sum(out=lsum, in_=lp[:, :nch], axis=AX)
                    rl = stats.tile([P, 1], f32, tag="rl")
                    nc.vector.reciprocal(rl, lsum)
                    ot = outp.tile([P, D], f32, tag="ot")
                    nc.vector.tensor_scalar_mul(ot, hres[:, i, :], rl)
                    nc.sync.dma_start(out=out2[hd, its, :], in_=ot)

                for i in tile_order:
                    nch = i // 4 + 1
                    its = slice(i * P, (i + 1) * P)
                    dcol = (i % 4) * P       # diag block start within last chunk
                    wlast = dcol + P         # causal width of last chunk
                    # f32r matmuls need moving dim >= 256 for full rate; widen
                    # (extra cols never read out of PSUM)
                    w_mm = max(wlast, 2 * P) if s2_r else wlast
                    jlast = nch - 1

                    # diag chunk first: its matmuls feed the bias every other
                    # chunk's exp needs
                    psD = psum.tile([P, SCH], f32, tag="ps")
                    k = 0
                    for lhs_src, rhs_src in s2_passes:
                        for ec in range(ND):
                            nc.tensor.matmul(
                                psD[:, :w_mm],
                                lhs_src[:, ec, its],
                                rhs_src[ec][jlast][:, :w_mm],
                                start=(k == 0), stop=(k == n2 - 1),
                            )
                            k += 1
                    sc = scs.tile([P, SCH], f32, tag="sc")
                    nc.vector.tensor_copy(sc[:, :wlast], psD[:, :wlast])
                    # diag extraction: mul by identity + negated row-sum
                    # (tensor_tensor_reduce crashes the device on this
                    # toolchain even with SBUF-only operands).  The mul and
                    # the causal-mask add run on the otherwise-idle Pool
                    # (gpsimd) engine; the X-axis reduce must stay on DVE.
                    dscr = stats.tile([P, P], f32, tag="dscr")
                    nc.vector.tensor_mul(dscr, sc[:, dcol : dcol + P], ident)
                    negdiag = stats.tile([P, 1], f32, tag="nd")
                    nc.vector.reduce_sum(
                        out=negdiag, in_=dscr, axis=AX, negate=True
                    )
                    nc.gpsimd.tensor_add(
                        sc[:, dcol : dcol + P], sc[:, dcol : dcol + P], cmask
                    )
                    lp = stats.tile([P, 4], f32, tag="lp")
                    for j in range(nch - 1):
                        ps = psum.tile([P, SCH], f32, tag="ps")
                        k = 0
                        for lhs_src, rhs_src in s2_passes:
                            for ec in range(ND):
                                nc.tensor.matmul(
                                    ps,
                                    lhs_src[:, ec, its],
                                    rhs_src[ec][j],
                                    start=(k == 0), stop=(k == n2 - 1),
                                )
                                k += 1
                        ex = escr.tile([P, SCH], f32, tag="ex")
                        nc.scalar.activation(
                            out=ex, in_=ps, func=EXP,
                            bias=negdiag, scale=1.0,
                            accum_out=lp[:, j : j + 1],
                        )
                    exd = escr.tile([P, SCH], f32, tag="ex")
                    nc.scalar.activation(
                        out=exd[:, :wlast], in_=sc[:, :wlast], func=EXP,
                        bias=negdiag, scale=1.0,
                        accum_out=lp[:, jlast : jlast + 1],
                    )

                    if pend is not None:
                        flush(pend)
                    pend = (lp, nch, i)
                flush(pend)

    nc.compile()
    return nc


_NC_CACHE = {}


def _get_nc(stage1=None, stage2=None):
    key = (stage1 or STAGE1, stage2 or STAGE2)
    if key not in _NC_CACHE:
        _NC_CACHE[key] = build_nc(*key)
    return _NC_CACHE[key]


def _consts():
    cmask = np.triu(np.full((P, P), NEG, np.float32), 1)
    ident = np.eye(P, dtype=np.float32)
    return cmask, ident


def make_in_maps(h, A):
    h = np.ascontiguousarray(h, dtype=np.float32)
    A = np.ascontiguousarray(A, dtype=np.float32)
    cmask, ident = _consts()
    in_maps = []
    for c in range(NCORES):
        b = c // 4
        h0 = 2 * (c % 4)
        in_maps.append({"hb": h[b], "A2": np.ascontiguousarray(A[h0 : h0 + 2]),
                        "cmaskd": cmask, "identd": ident})
    return in_maps


def assemble(results):
    full = np.empty((B, H, T, D), dtype=np.float32)
    for c in range(NCORES):
        b = c // 4
        h0 = 2 * (c % 4)
        o = results[c]["out2"]
        full[b, h0] = o[0]
        full[b, h0 + 1] = o[1]
    return full.reshape(B, T, H * D)


def kernel(h, A):
    nc = _get_nc()
    res = bass_utils.run_bass_kernel_spmd(
        nc, make_in_maps(h, A), core_ids=list(range(NCORES))
    )
    return assemble(res.results)


# revision 30
# speedup vs baseline: 1.0321x; 1.0078x over previous
"""Trainium2 Bass kernel for causal bilinear self-attention (diagonal variant).

Computes, per (b, head):
    scores[t, s] = h[b, t] @ A[head] @ h[b, s]        (causal: s <= t)
    attn = softmax(scores, axis=-1)
    out[b, head, t, :] = attn[t, t] * h[b, t, :]
returned reshaped row-major to (B, T, H*d)  (faithful torch .view semantics).

Only the diagonal of the attention matrix is needed:
    attn[t, t] = exp(scores[t,t] - m) / sum_{s<=t} exp(scores[t,s] - m)
               = 1 / sum_{s<=t} exp(scores[t,s] - scores[t,t])
Using bias = -scores[t,t] inside the exp (instead of the row max) keeps the
denominator in [1, inf) -- the s==t term is exactly exp(0) -- so NO row-max
pass is needed: overflow to inf gives reciprocal 0, matching the true
underflowed attention weight.  This removes the reduce_max over the whole
causal triangle and the exp-diagonal extraction, which made the vector
engine the bottleneck (86% busy vs PE 64%) in the max-subtracting version.

Engine budget per core (r1, cost model): PE ~92 us (bound), DVE ~80 us,
ACT ~59 us.  Layout/scheduling choices:
  - h[b] stays resident in SBUF (4 MB): loaded once, reused for transposes
    and the final attn*h scale (no reload).
  - A DMA'd per head so stage 1 of head 0 isn't gated on both heads' A.
  - hT via PE transpose with f32r-tagged input (1.5 cyc/row vs 2.0 for f32;
    the f32r mantissa rounding is applied by the PE at matmul read anyway).
  - head 0 walks row tiles ascending, head 1 descending, with stage-1
    t-slices ordered to match, so the final row tile is the 1-chunk tile 0
    (short epilogue tail instead of the 4-chunk tile 15).
  - per row tile, the diagonal chunk's matmuls are emitted FIRST so the
    bias (-scores[t,t]) is ready before the other chunks' exps; the
    lsum/reciprocal/scale epilogue of tile k is emitted after tile k+1's
    exps so the in-order DVE never stalls waiting on ACT.

Precision: TensorE fp32 costs 4 cyc/row; float32r (TF32-like, ~11-bit
mantissa) costs 1 cyc/row at moving dim >= 256.  "r1" = single f32r pass
per stage (measured 3.4e-3 rel err on HW vs the 2e-2 gate).  "r2" adds a
residual pass for A (stage 1) / g (stage 2); "f32" is the exact path.

Hardware notes (found empirically on this axon/neuronxcc toolchain):
  - tensor_tensor_reduce with a PSUM input crashes the device; so does an
    ACT read of a PSUM region modified in place by the DVE.  PSUM is
    written only by the PE; DVE copy/reduce-class ops and ACT activations
    may read it; two-operand DVE ops only run on SBUF.
  - mask constants are DMA'd from host inputs (no gpsimd affine_select).

Sharding: 16 (b, head) pairs across 8 cores -> core c handles b = c // 4,
heads 2*(c%4) and 2*(c%4)+1.
"""

import os
import sys

try:
    import concourse.bass  # noqa: F401
except ImportError:  # pragma: no cover
    sys.path.insert(0, "/opt/trn_rl_repo")

import numpy as np

import concourse.bass as bass  # noqa: F401
import concourse.tile as tile
from concourse import bacc, bass_utils, mybir

B, T, D, H = 2, 2048, 512, 8
NCORES = 8
P = 128
NT = T // P      # 16 row tiles
ND = D // P      # 4 contraction chunks
SCH = 512        # s-chunk width (one PSUM bank of fp32)
NS = T // SCH    # 4 column slices of hT
NEG = -1.0e30

f32 = mybir.dt.float32
f32r = mybir.dt.float32r

STAGE1 = os.environ.get("BK_STAGE1", "r1")
STAGE2 = os.environ.get("BK_STAGE2", "r1")


def build_nc(stage1=None, stage2=None):
    stage1 = stage1 or STAGE1
    stage2 = stage2 or STAGE2
    assert stage1 in ("f32", "r1", "r2") and stage2 in ("f32", "r1", "r2")
    s1_r = stage1 != "f32"
    s2_r = stage2 != "f32"
    need_hT32 = stage1 == "f32" or stage2 == "f32"
    need_hTr = s1_r or s2_r

    nc = bacc.Bacc("TRN2", target_bir_lowering=False, debug=False)
    hb = nc.dram_tensor("hb", [T, D], f32, kind="ExternalInput")
    A2 = nc.dram_tensor("A2", [2, D, D], f32, kind="ExternalInput")
    cmaskd = nc.dram_tensor("cmaskd", [P, P], f32, kind="ExternalInput")
    identd = nc.dram_tensor("identd", [P, P], f32, kind="ExternalInput")
    out2 = nc.dram_tensor("out2", [2, T, D], f32, kind="ExternalOutput")
    hb_t = hb[:].rearrange("(n p) d -> p n d", p=P)  # [128, 16, 512] view

    AX = mybir.AxisListType.X
    EXP = mybir.ActivationFunctionType.Exp

    with tile.TileContext(nc) as tc:
        with (
            tc.tile_pool(name="const", bufs=1) as constp,
            tc.tile_pool(name="big", bufs=1) as big,
            tc.tile_pool(name="gpool", bufs=1) as gpool,
            tc.tile_pool(name="psum", bufs=8, space="PSUM") as psum,
            tc.tile_pool(name="scs", bufs=4) as scs,
            tc.tile_pool(name="escr", bufs=4) as escr,
            tc.tile_pool(name="stats", bufs=16) as stats,
            tc.tile_pool(name="outp", bufs=4) as outp,
        ):
            ident = constp.tile([P, P], f32)
            nc.gpsimd.dma_start(out=ident, in_=identd[:])
            cmask = constp.tile([P, P], f32)
            nc.gpsimd.dma_start(out=cmask, in_=cmaskd[:])

            # h resident in SBUF: hres[p, i, d] = h[i*128 + p, d].  Tile 0 is
            # split per 128-col chunk so the first transpose starts sooner,
            # and tiles round-robin over three DGE queues so arrival is not
            # paced by one queue's dispatch rate.  A's per-head DMAs slot
            # into the gpsimd queue: A0 early enough to be rounded before
            # stage 1 needs it (~10us), A1 at the back (needed ~45us).
            A_sb = big.tile([P, 2, ND, D], f32)
            hres = big.tile([P, NT, D], f32)
            dmaq = [nc.sync, nc.scalar, nc.gpsimd]
            for c in range(ND):
                dmaq[c % 3].dma_start(
                    out=hres[:, 0, c * P : (c + 1) * P],
                    in_=hb_t[:, 0, c * P : (c + 1) * P],
                )
            for i in range(1, NT):
                dmaq[i % 3].dma_start(out=hres[:, i, :], in_=hb_t[:, i, :])
            for hd in range(2):
                nc.gpsimd.dma_start(
                    out=A_sb[:, hd],
                    in_=A2[hd].rearrange("(c p) e -> p c e", p=P),
                )
            if s1_r:
                # BIR verifier requires f32r matmul inputs to come from an
                # op that rounds to f32r, so an explicit rounded copy (a
                # bitcast view of the DMA'd f32 tile is rejected).  The
                # copies run on the otherwise-idle Pool engine so the DVE
                # hT drains aren't delayed behind them.
                A_r = big.tile([P, 2, ND, D], f32r)
                for hd_ in range(2):
                    for dc in range(ND):
                        nc.gpsimd.tensor_copy(
                            A_r[:, hd_, dc], A_sb[:, hd_, dc]
                        )

            def a_hi(hd, dc, ecs):
                if s1_r:
                    return A_r[:, hd, dc, ecs]
                return A_sb[:, hd, dc, ecs]

            # h^T: hT*[p, c, t] = h[t, c*128 + p], via PE transpose
            def _mk(name, dt_):
                return [[big.tile([P, SCH], dt_, name=f"{name}_{c}_{s}")
                         for s in range(NS)] for c in range(ND)]
            hT32 = _mk("hT32", f32) if need_hT32 else None
            hTr = _mk("hTr", f32r) if need_hTr else None

            def _hT(tens, c, lo, width):
                s, off = lo // SCH, lo % SCH
                return tens[c][s][:, off : off + width]

            need_g32 = stage2 == "f32"

            if stage1 == "f32":
                s1_rhs = [hT32]
            else:
                s1_rhs = [hTr] if stage1 == "r1" else [hTr, hTr]
            n1 = (2 if stage1 == "r2" else 1) * ND

            def emit_stage1_tsl(hd, tsl, g32, gh, gl):
                ts_ = slice(tsl * SCH, (tsl + 1) * SCH)
                for ec in range(ND):
                    ecs = slice(ec * P, (ec + 1) * P)
                    pg = psum.tile([P, SCH], f32, tag="ps")
                    k = 0
                    for ip in range(2 if stage1 == "r2" else 1):
                        for dc in range(ND):
                            lhs = (A_l[:, hd, dc, ecs] if (stage1 == "r2" and ip == 1)
                                   else a_hi(hd, dc, ecs))
                            nc.tensor.matmul(
                                pg, lhs, s1_rhs[ip][dc][tsl],
                                start=(k == 0), stop=(k == n1 - 1),
                            )
                            k += 1
                    if g32 is not None:
                        nc.vector.tensor_copy(g32[:, ec, ts_], pg)
                    if gh is not None:
                        nc.vector.tensor_copy(gh[:, ec, ts_], pg)
                    if gl is not None:
                        nc.vector.tensor_sub(
                            gl[:, ec, ts_], pg, gh[:, ec, ts_].bitcast(f32)
                        )

            def alloc_g():
                g32 = gpool.tile([P, ND, T], f32, tag="g32", name="gT32") if need_g32 else None
                gh = gpool.tile([P, ND, T], f32r, tag="gh", name="gTh") if s2_r else None
                gl = gpool.tile([P, ND, T], f32r, tag="gl", name="gTl") if stage2 == "r2" else None
                return g32, gh, gl

            for i in range(NT):
                for c in range(ND):
                    src = hres[:, i, c * P : (c + 1) * P]
                    pt = psum.tile([P, P], f32, tag="ps")
                    nc.tensor.transpose(pt, src, ident)
                    if need_hT32:
                        nc.vector.tensor_copy(_hT(hT32, c, i * P, P), pt)
                    if need_hTr:
                        nc.vector.tensor_copy(_hT(hTr, c, i * P, P), pt)
            if stage1 == "r2":
                A_l = big.tile([P, 2, ND, D], f32r)
                nc.vector.tensor_sub(A_l, A_sb, A_r.bitcast(f32))

            g_head0 = alloc_g()
            for tsl in range(NS):
                emit_stage1_tsl(0, tsl, *g_head0)

            for hd in range(2):
                # ascending everywhere: the big late tiles keep the PE busy
                # while earlier tiles' epilogue chains drain.  Head 1 ends on
                # small tile 3 so only a short chain trails the last matmul.
                if hd == 0:
                    gT32, gTh, gTl = g_head0
                    tile_order = list(range(NT))
                else:
                    gT32, gTh, gTl = alloc_g()
                    for tsl in range(NS):
                        emit_stage1_tsl(1, tsl, gT32, gTh, gTl)
                    tile_order = [0, 1, 2] + list(range(4, NT)) + [3]

                if stage2 == "f32":
                    s2_passes = [(gT32, hT32)]
                elif stage2 == "r1":
                    s2_passes = [(gTh, hTr)]
                else:
                    s2_passes = [(gTh, hTr), (gTl, hTr)]
                n2 = len(s2_passes) * ND

                # ---- stage 2 + diag-bias softmax, per row tile ----
                pend = None  # deferred epilogue: (lp, nch, i)

                def flush(pend):
                    lp, nch, i = pend
                    its = slice(i * P, (i + 1) * P)
                    lsum = stats.tile([P, 1], f32, tag="ls")
                    nc.vector.reduce_sum(out=lsum, in_=lp[:, :nch], axis=AX)
                    rl = stats.tile([P, 1], f32, tag="rl")
                    nc.vector.reciprocal(rl, lsum)
                    ot = outp.tile([P, D], f32, tag="ot")
                    nc.vector.tensor_scalar_mul(ot, hres[:, i, :], rl)
                    nc.sync.dma_start(out=out2[hd, its, :], in_=ot)

                for i in tile_order:
                    nch = i // 4 + 1
                    its = slice(i * P, (i + 1) * P)
                    dcol = (i % 4) * P       # diag block start within last chunk
                    wlast = dcol + P         # causal width of last chunk
                    # f32r matmuls need moving dim >= 256 for full rate; widen
                    # (extra cols never read out of PSUM)
                    w_mm = max(wlast, 2 * P) if s2_r else wlast
                    jlast = nch - 1

                    # diag chunk first: its matmuls feed the bias every other
                    # chunk's exp needs
                    psD = psum.tile([P, SCH], f32, tag="ps")
                    k = 0
                    for lhs_src, rhs_src in s2_passes:
                        for ec in range(ND):
                            nc.tensor.matmul(
                                psD[:, :w_mm],
                                lhs_src[:, ec, its],
                                rhs_src[ec][jlast][:, :w_mm],
                                start=(k == 0), stop=(k == n2 - 1),
                            )
                            k += 1
                    sc = scs.tile([P, SCH], f32, tag="sc")
                    nc.vector.tensor_copy(sc[:, :wlast], psD[:, :wlast])
                    # diag extraction: mul by identity + negated row-sum
                    # (tensor_tensor_reduce crashes the device on this
                    # toolchain even with SBUF-only operands).  The mul and
                    # the causal-mask add run on the otherwise-idle Pool
                    # (gpsimd) engine; the X-axis reduce must stay on DVE.
                    dscr = stats.tile([P, P], f32, tag="dscr")
                    nc.vector.tensor_mul(dscr, sc[:, dcol : dcol + P], ident)
                    negdiag = stats.tile([P, 1], f32, tag="nd")
                    nc.vector.reduce_sum(
                        out=negdiag, in_=dscr, axis=AX, negate=True
                    )
                    nc.gpsimd.tensor_add(
                        sc[:, dcol : dcol + P], sc[:, dcol : dcol + P], cmask
                    )
                    lp = stats.tile([P, 4], f32, tag="lp")
                    for j in range(nch - 1):
                        ps = psum.tile([P, SCH], f32, tag="ps")
                        k = 0
                        for lhs_src, rhs_src in s2_passes:
                            for ec in range(ND):
                                nc.tensor.matmul(
                                    ps,
                                    lhs_src[:, ec, its],
                                    rhs_src[ec][j],
                                    start=(k == 0), stop=(k == n2 - 1),
                                )
                                k += 1
                        ex = escr.tile([P, SCH], f32, tag="ex")
                        nc.scalar.activation(
                            out=ex, in_=ps, func=EXP,
                            bias=negdiag, scale=1.0,
                            accum_out=lp[:, j : j + 1],
                        )
                    exd = escr.tile([P, SCH], f32, tag="ex")
                    nc.scalar.activation(
                        out=exd[:, :wlast], in_=sc[:, :wlast], func=EXP,
                        bias=negdiag, scale=1.0,
                        accum_out=lp[:, jlast : jlast + 1],
                    )

                    if pend is not None:
                        flush(pend)
                    pend = (lp, nch, i)
                flush(pend)

    nc.compile()
    return nc


_NC_CACHE = {}


def _get_nc(stage1=None, stage2=None):
    key = (stage1 or STAGE1, stage2 or STAGE2)
    if key not in _NC_CACHE:
        _NC_CACHE[key] = build_nc(*key)
    return _NC_CACHE[key]


def _consts():
    cmask = np.triu(np.full((P, P), NEG, np.float32), 1)
    ident = np.eye(P, dtype=np.float32)
    return cmask, ident


def make_in_maps(h, A):
    h = np.ascontiguousarray(h, dtype=np.float32)
    A = np.ascontiguousarray(A, dtype=np.float32)
    cmask, ident = _consts()
    in_maps = []
    for c in range(NCORES):
        b = c // 4
        h0 = 2 * (c % 4)
        in_maps.append({"hb": h[b], "A2": np.ascontiguousarray(A[h0 : h0 + 2]),
                        "cmaskd": cmask, "identd": ident})
    return in_maps


def assemble(results):
    full = np.empty((B, H, T, D), dtype=np.float32)
    for c in range(NCORES):
        b = c // 4
        h0 = 2 * (c % 4)
        o = results[c]["out2"]
        full[b, h0] = o[0]
        full[b, h0 + 1] = o[1]
    return full.reshape(B, T, H * D)


def kernel(h, A):
    nc = _get_nc()
    res = bass_utils.run_bass_kernel_spmd(
        nc, make_in_maps(h, A), core_ids=list(range(NCORES))
    )
    return assemble(res.results)
